# revision 34
# baseline (speedup 1.0000x reference)
"""Causal self-attention on 8 TRN2 NeuronCores.

Sharding: core c -> (batch b = c//2, head-group g = c%2).
B=4, T=2048, D=1024, 16 heads x 64. Each core computes attention for its
batch and its 8 heads, plus the partial output projection for those heads;
the host sums the two partial projections per batch.

Perf-oriented restructure vs the phased baseline:
  - Inputs host-packed into a handful of large, DMA-friendly tensors
    (x halves / wqk / wv as bf16, wp as f32) split across both HWDGE
    rings (SP + Activation) so compute starts ~6us in instead of ~38us.
  - One flat pool scope; tiles split per (half / chunk) so the Tile list
    scheduler can overlap QKV projection, attention, and the output
    projection.  This keeps the PE matmul stream dense, which keeps the
    PE_HAM clock gate at 8/8 (2.4 GHz) instead of oscillating to 4/8.
  - Emission order (= scheduler priority): proj half-0; then per head
    {attn c=0, proj-half1 m-group, attn c=1, v-group}; then per head
    {attn c=2, out-proj col 0}; {attn c=3, out-proj col 1}; cols 2,3.
    ACT does only exp (the phase-2 bottleneck); all PSUM evacuation is
    on DVE; normalization uses reciprocal_approx_fast + GpSimd
    partition-broadcast.
Attention is computed in S^T orientation (k on partitions, q on free dim)
with the ones-column trick producing softmax denominators inside the AV
accumulation (av row 64).
"""

import sys

for _p in ("/opt/pypackages", "/opt/trn_rl_repo"):
    if _p not in sys.path:
        sys.path.insert(0, _p)

from contextlib import ExitStack

import ml_dtypes
import numpy as np

import concourse.bass as bass
import concourse.tile as tile
from concourse import bacc, mybir
from concourse.bass_utils import run_bass_kernel_spmd

F32 = mybir.dt.float32
F32R = mybir.dt.float32r
BF16 = mybir.dt.bfloat16
AF = mybir.ActivationFunctionType
OP = mybir.AluOpType

D = 1024
T = 2048
NH_LOC = 8          # heads per core
DH = 64
GF = NH_LOC * DH    # 512 features per group

LAST_RESULTS = None
_CACHED = None


def build_program(dbg=False):
    nc = bacc.Bacc("TRN2", target_bir_lowering=False, debug=False)

    # x: [128, 16384] bf16; half h, k-block k, col t -> [:, h*8192 + k*1024 + t]
    x_d = nc.dram_tensor("xH", [128, 2 * 8192], BF16, kind="ExternalInput").ap()
    # wqk: [128, 8192] bf16; m-group m, k-block k -> [:, m*1024 + k*128 : +128]
    wqk_d = nc.dram_tensor("wqkP", [128, 8192], BF16, kind="ExternalInput").ap()
    # wv: [128, 4096] bf16; k-block k -> [:, k*512 : +512]
    wv_d = nc.dram_tensor("wvP", [128, 4096], BF16, kind="ExternalInput").ap()
    # wp: [128, 4096] f32; kk-block, out-col m -> [:, kk*1024 + m*128 : +128]
    wp_d = nc.dram_tensor("wpP", [128, 4096], F32R, kind="ExternalInput").ap()
    mask_d = nc.dram_tensor("mask", [128, 2048], BF16, kind="ExternalInput").ap()
    yT_d = nc.dram_tensor("yT", [D, T], BF16, kind="ExternalOutput").ap()
    if dbg:
        dqk_d = nc.dram_tensor("dqk", [128, 16 * 1024], BF16, kind="ExternalOutput").ap()
        dv_d = nc.dram_tensor("dv", [128, 16 * 520], BF16, kind="ExternalOutput").ap()
        dot_d = nc.dram_tensor("dot", [128, 16 * 512], F32R, kind="ExternalOutput").ap()

    with tile.TileContext(nc) as tc:
        with ExitStack() as octx:
            p_pool = octx.enter_context(tc.tile_pool(name="persist", bufs=1))
            pt_pool = octx.enter_context(tc.tile_pool(name="pt", bufs=6))
            y_pool = octx.enter_context(tc.tile_pool(name="y", bufs=6))
            r_pool = octx.enter_context(tc.tile_pool(name="recip", bufs=4))
            yp_pool = octx.enter_context(tc.tile_pool(name="ypart", bufs=1))
            ps_s = octx.enter_context(tc.tile_pool(name="ps_s", bufs=3, space="PSUM"))
            ps_av = octx.enter_context(tc.tile_pool(name="ps_av", bufs=2, space="PSUM"))

            # ---- persistent tiles -------------------------------------
            x_t = [p_pool.tile([128, 8192], BF16, name=f"x{h}", tag=f"x{h}") for h in range(2)]
            wqk_t = p_pool.tile([128, 8192], BF16, name="wqk", tag="wqk")
            wv_t = p_pool.tile([128, 4096], BF16, name="wv", tag="wv")
            wp_t = p_pool.tile([128, 4096], F32R, name="wp", tag="wp")
            mask_t = p_pool.tile([128, 2048], BF16, name="mask", tag="mask")
            # q/k features: qk[m][half]; m 0..3 q-feats, 4..7 k-feats
            qk_t = [[p_pool.tile([128, 1024], BF16, name=f"qk{m}_{h}", tag=f"qk{m}_{h}")
                     for h in range(2)] for m in range(8)]
            # v natural: 16 tiles [128, 520]; cols h*65+0..63 v-feats, col h*65+64 ones
            v_t = [p_pool.tile([128, 8 * (DH + 1)], BF16, name=f"v{t}", tag=f"v{t}")
                   for t in range(16)]
            # normalized attention out: ot[qm][c] [128,512]; rows 0..63 head 2qm
            ot_t = [[p_pool.tile([128, 512], F32R, name=f"ot{qm}_{c}", tag=f"ot{qm}_{c}")
                     for c in range(4)] for qm in range(4)]

            # ---- input DMAs (two HWDGE rings, ordered by first use) ---
            # ring A (sync): x0 k0-3, wqk m1/m5, x1; ring B (scalar):
            # wqk m0/m4, x0 k4-7, wv, wqk m2/m6/m3/m7, mask, wp
            def dma_wqk(eng, m):
                eng.dma_start(wqk_t[:, m * 1024:(m + 1) * 1024],
                              wqk_d[:, m * 1024:(m + 1) * 1024])

            nc.sync.dma_start(x_t[0][:, 0:512], x_d[:, 0:512])
            nc.scalar.dma_start(wqk_t[:, 0:256], wqk_d[:, 0:256])
            nc.sync.dma_start(x_t[0][:, 512:1024], x_d[:, 512:1024])
            nc.scalar.dma_start(wqk_t[:, 256:1024], wqk_d[:, 256:1024])
            nc.sync.dma_start(x_t[0][:, 1024:2048], x_d[:, 1024:2048])
            dma_wqk(nc.scalar, 4)
            nc.sync.dma_start(x_t[0][:, 2048:4096], x_d[:, 2048:4096])
            nc.scalar.dma_start(x_t[0][:, 4096:6144], x_d[:, 4096:6144])
            nc.sync.dma_start(x_t[0][:, 6144:8192], x_d[:, 6144:8192])
            nc.scalar.dma_start(wv_t[:], wv_d[:])
            dma_wqk(nc.sync, 1)
            dma_wqk(nc.sync, 5)
            dma_wqk(nc.scalar, 2)
            dma_wqk(nc.scalar, 6)
            dma_wqk(nc.scalar, 3)
            dma_wqk(nc.scalar, 7)
            nc.scalar.dma_start(mask_t[:], mask_d[:])
            nc.sync.dma_start(x_t[1][:], x_d[:, 8192:16384])
            nc.scalar.dma_start(wp_t[:], wp_d[:])

            # ones column for the softmax-denominator trick
            for t in range(16):
                ones_ap = v_t[t][:].rearrange("p (h e) -> p h e", h=8, e=65)[:, :, 64:65]
                nc.vector.memset(ones_ap, 1.0)

            # ---- helpers ---------------------------------------------
            def m_group(half, m):
                """q/k features m*128..+128 for T-cols half*1024..+1024."""
                psm = ps_s.tile([128, 1024], F32, name="s", tag="s")
                for k in range(8):
                    wt = wqk_t[:, m * 1024 + k * 128: m * 1024 + (k + 1) * 128]
                    for n in range(2):
                        nc.tensor.matmul(
                            psm[:, n * 512:(n + 1) * 512], wt,
                            x_t[half][:, k * 1024 + n * 512: k * 1024 + (n + 1) * 512],
                            start=(k == 0), stop=(k == 7),
                            skip_group_check=True,
                        )
                nc.scalar.activation(qk_t[m][half][:], psm[:], AF.Copy)

            def v_group(half, tt):
                """v rows (t-positions) half*1024 + tt*128..+128, all 8 heads."""
                psv = ps_s.tile([128, 1024], F32, name="s", tag="s")[:, 0:512]
                for k in range(8):
                    nc.tensor.matmul(
                        psv[:],
                        x_t[half][:, k * 1024 + tt * 128: k * 1024 + (tt + 1) * 128],
                        wv_t[:, k * 512:(k + 1) * 512],
                        start=(k == 0), stop=(k == 7),
                        skip_group_check=True,
                    )
                vt = v_t[half * 8 + tt]
                src = psv[:].rearrange("p (h e) -> p h e", h=8, e=64)
                dst = vt[:].rearrange("p (h e) -> p h e", h=8, e=65)[:, :, 0:64]
                nc.vector.tensor_copy(dst, src)

            def job(h, c):
                """Attention for head h, query chunk c (512 queries)."""
                qm = h // 2
                qoff = 64 * (h % 2)
                npieces = 4 * c + 4
                qT = qk_t[qm][c // 2][qoff:qoff + 64,
                                      (c % 2) * 512:(c % 2) * 512 + 512]
                av = ps_av.tile([65, 512], F32, name="av", tag="av")
                for p in range(npieces // 2):
                    # For a diagonal block a (kpos c*512+a*128..), query cols
                    # < a*128 are entirely above the diagonal: S / mask / AV
                    # all skip them (exp still covers the full tile; the
                    # garbage cols are never masked nor accumulated).
                    offs = []
                    for idx in range(2):
                        j = 2 * p + idx
                        offs.append((j - 4 * c) * 128 if j // 4 == c else 0)
                    s = ps_s.tile([128, 1024], F32, name="s", tag="s")
                    for idx in range(2):
                        j = 2 * p + idx
                        off = offs[idx]
                        kT = qk_t[4 + qm][j // 8][qoff:qoff + 64,
                                                  (j % 8) * 128:(j % 8 + 1) * 128]
                        nc.tensor.matmul(
                            s[:, idx * 512 + off:(idx + 1) * 512], kT,
                            qT[:, off:512],
                            start=True, stop=True,
                            skip_group_check=True,
                        )
                    pt = pt_pool.tile([128, 1024], BF16, name="pt", tag="pt")
                    lo = offs[0]
                    nc.scalar.activation(pt[:, lo:1024], s[:, lo:1024],
                                         AF.Exp, scale=0.125)
                    for idx in range(2):
                        j = 2 * p + idx
                        off = offs[idx]
                        if j // 4 == c:  # diagonal block -> causal mask
                            pp = (j * 128 - c * 512) // 128
                            nc.vector.tensor_tensor(
                                pt[:, idx * 512 + off:(idx + 1) * 512],
                                pt[:, idx * 512 + off:(idx + 1) * 512],
                                mask_t[:, pp * 512 + off:(pp + 1) * 512],
                                op=OP.mult,
                            )
                    for idx in range(2):
                        j = 2 * p + idx
                        off = offs[idx]
                        nc.tensor.matmul(
                            av[:, off:512],
                            v_t[j][:, h * 65:(h + 1) * 65],
                            pt[:, idx * 512 + off:(idx + 1) * 512],
                            start=(j == 0), stop=(j == npieces - 1),
                            skip_group_check=True,
                        )
                # normalize + evacuate into ot
                den = r_pool.tile([1, 512], F32, name="den", tag="den")
                nc.vector.tensor_copy(den[:], av[64:65, :])
                rec = r_pool.tile([1, 512], F32, name="rec", tag="rec")
                nc.vector.reciprocal_approx_fast(rec[:], den[:])
                rb = r_pool.tile([64, 512], F32, name="rb", tag="rb")
                nc.gpsimd.partition_broadcast(rb[:], rec[:])
                nc.vector.tensor_tensor(
                    ot_t[qm][c][qoff:qoff + 64, :],
                    av[0:64, :], rb[:], op=OP.mult,
                )

            def ph3(n, m):
                """Output projection: y rows m*128..+128, cols n*512..+512."""
                psy = ps_s.tile([128, 1024], F32, name="s", tag="s")[:, 0:512]
                for kk in range(4):
                    nc.tensor.matmul(
                        psy[:],
                        wp_t[:, kk * 1024 + m * 128: kk * 1024 + (m + 1) * 128],
                        ot_t[kk][n][:],
                        start=(kk == 0), stop=(kk == 3),
                        skip_group_check=True,
                    )
                yt = y_pool.tile([128, 512], BF16, name="yst", tag="yst")
                nc.vector.tensor_copy(yt[:], psy[:])
                eng = nc.scalar if (n == 3 and m % 2 == 1) else nc.sync
                eng.dma_start(
                    yT_d[m * 128:(m + 1) * 128, n * 512:(n + 1) * 512], yt[:]
                )

            yp_t = [yp_pool.tile([128, 512], BF16, name=f"yp{m}", tag=f"yp{m}")
                    for m in range(8)]

            def ph3a(m):
                """Out-proj col 3, partial contraction kk=0..2 -> SBUF."""
                psy = ps_s.tile([128, 1024], F32, name="s", tag="s")[:, 0:512]
                for kk in range(3):
                    nc.tensor.matmul(
                        psy[:],
                        wp_t[:, kk * 1024 + m * 128: kk * 1024 + (m + 1) * 128],
                        ot_t[kk][3][:],
                        start=(kk == 0), stop=(kk == 2),
                        skip_group_check=True,
                    )
                nc.vector.tensor_copy(yp_t[m][:], psy[:])

            def ph3b(m):
                """Out-proj col 3: last kk + add partial, store."""
                psy = ps_s.tile([128, 1024], F32, name="s", tag="s")[:, 0:512]
                nc.tensor.matmul(
                    psy[:],
                    wp_t[:, 3 * 1024 + m * 128: 3 * 1024 + (m + 1) * 128],
                    ot_t[3][3][:],
                    start=True, stop=True,
                    skip_group_check=True,
                )
                yt = y_pool.tile([128, 512], BF16, name="yst", tag="yst")
                nc.vector.tensor_tensor(yt[:], psy[:], yp_t[m][:], op=OP.add)
                eng = nc.scalar if m % 2 == 1 else nc.sync
                eng.dma_start(
                    yT_d[m * 128:(m + 1) * 128, 3 * 512:4 * 512], yt[:]
                )

            # ---- emission (priority) order ---------------------------
            # S1: projection over T-half 0
            for i in range(4):
                m_group(0, i)
                m_group(0, 4 + i)
                v_group(0, i)
            for tt in range(4, 8):
                v_group(0, tt)
            # S2: attn chunks 0,1 zipped with projection half 1.
            # k-features first, and chunk-2 jobs pulled in as soon as their
            # kT blocks / v tiles exist, to spread exp (ACT) work evenly.
            m2_order = [4, 0, 5, 1, 6, 2, 7, 3]
            for h in range(NH_LOC):
                job(h, 0)
                m_group(1, m2_order[h])
                job(h, 1)
                v_group(1, h)
                if h >= 3:
                    job(h - 3, 2)
            # S3: remaining chunk-2, all chunk-3, out-proj interleaved
            job(5, 2)
            job(0, 3)
            for m in range(4):
                ph3(0, m)
            job(6, 2)
            job(1, 3)
            for m in range(4, 8):
                ph3(0, m)
            job(7, 2)
            job(2, 3)
            for m in range(4):
                ph3(1, m)
            job(3, 3)
            for m in range(4, 8):
                ph3(1, m)
            job(4, 3)
            for m in range(4):
                ph3(2, m)
            job(5, 3)
            for m in range(4, 8):
                ph3(2, m)
            job(6, 3)
            for m in range(4):
                ph3a(m)
            job(7, 3)
            for m in range(4, 8):
                ph3a(m)
            for m in range(8):
                ph3b(m)

            if dbg:
                for m in range(8):
                    for half in range(2):
                        i = m * 2 + half
                        nc.sync.dma_start(
                            dqk_d[:, i * 1024:(i + 1) * 1024], qk_t[m][half][:]
                        )
                for t in range(16):
                    nc.sync.dma_start(dv_d[:, t * 520:(t + 1) * 520], v_t[t][:])
                for qm in range(4):
                    for c in range(4):
                        i = qm * 4 + c
                        nc.sync.dma_start(
                            dot_d[:, i * 512:(i + 1) * 512], ot_t[qm][c][:]
                        )

    nc.compile()
    return nc


def _make_mask():
    mask = np.zeros((128, 2048), dtype=np.float32)
    kk = np.arange(128)[:, None]
    q = np.arange(512)[None, :]
    for p in range(4):
        d = 128 * p
        mask[:, p * 512:(p + 1) * 512] = ((q - d) >= kk).astype(np.float32)
    return mask


def kernel(x, w_qkv, w_proj):
    global LAST_RESULTS, _CACHED
    x = np.asarray(x, dtype=np.float32)
    w_qkv = np.asarray(w_qkv, dtype=np.float32)
    w_proj = np.asarray(w_proj, dtype=np.float32)
    B = x.shape[0]

    if _CACHED is None:
        _CACHED = build_program()
    nc = _CACHED

    mask = _make_mask().astype(ml_dtypes.bfloat16)
    in_maps = []
    for c in range(8):
        b, g = c // 2, c % 2
        # x packed: xH[p, half*8192 + k*1024 + t] = x[b].T[k*128+p, half*1024+t]
        xT = x[b].T.astype(ml_dtypes.bfloat16)            # [1024, 2048]
        xH = np.ascontiguousarray(
            xT.reshape(8, 128, 2, 1024).transpose(1, 2, 0, 3).reshape(128, 16384))
        # wqk packed: [p, m*1024 + k*128 + j] = wqkT[k*128+p, m*128+j]
        wq = w_qkv[g * GF:(g + 1) * GF, :]                # [512, 1024]
        wk = w_qkv[D + g * GF: D + (g + 1) * GF, :]
        wqkT = np.concatenate([wq, wk], axis=0).T         # [1024 k, 1024 m]
        wqkP = np.ascontiguousarray(
            wqkT.reshape(8, 128, 8, 128).transpose(1, 2, 0, 3).reshape(128, 8192)
        ).astype(ml_dtypes.bfloat16)
        # wv packed: [p, k*512 + f] = wvT[k*128+p, f]
        wv = w_qkv[2 * D + g * GF: 2 * D + (g + 1) * GF, :]
        wvT = wv.T                                        # [1024 k, 512 f]
        wvP = np.ascontiguousarray(
            wvT.reshape(8, 128, 512).transpose(1, 0, 2).reshape(128, 4096)
        ).astype(ml_dtypes.bfloat16)
        # wp packed: [p, kk*1024 + d] = wpT[kk*128+p, d]
        wpT = w_proj[:, g * GF:(g + 1) * GF].T            # [512 f, 1024 d]
        wpP = np.ascontiguousarray(
            wpT.reshape(4, 128, 1024).transpose(1, 0, 2).reshape(128, 4096))
        in_maps.append({
            "xH": xH,
            "wqkP": wqkP,
            "wvP": wvP,
            "wpP": wpP,
            "mask": mask,
        })

    res = run_bass_kernel_spmd(nc, in_maps, core_ids=list(range(8)))
    LAST_RESULTS = res

    y = np.empty_like(x)
    for b in range(B):
        yT = (res.results[2 * b]["yT"].astype(np.float32)
              + res.results[2 * b + 1]["yT"].astype(np.float32))
        y[b] = yT.T
    return y
